# revision 21
# baseline (speedup 1.0000x reference)
"""NT-Xent loss kernel for 8 TRN2 NeuronCores (Bass/Tile).

Math: with x = memory bank rows, the per-sample negative-similarity sum
    neg[n] = sum_m real_hat[n] . mem_hat[m] = real_hat[n] . s,
    s = sum_m mem[m] / ||mem[m]||
so the N x M GEMM collapses into a weighted row-sum of the memory bank
(one D-vector), which is purely memory-bound.

Distribution (8 cores, two SPMD launches, no collectives):
  - NEFF A: memory bank [65536,128] sharded by rows, 8192 rows/core. Each
    core computes row norms and the inv-norm-weighted row sum of its shard
    via packed float32r matmuls (M=8 "diagonal" trick + mask), emitting
    red[8,128] whose row-sum is the core's partial s.
  - Host: concatenates the 8 red blocks -> redall[64,128] (pure gather).
  - NEFF B: data-parallel CE over N. Every core reduces redall and
    broadcasts s to 128 partitions with one ones-matmul, then computes the
    stable softplus CE for its 128 samples and emits sum(loss)/N. The host
    sums the 8 partial scalars.
Cross-core start skew makes on-device collectives cost ~70us on this
runtime (44us entry barrier + ~20us mesh floor), so two launches are much
cheaper than one AllGather.
"""

import numpy as np

import concourse.bacc as bacc
import concourse.bass as bass
import concourse.mybir as mybir
import concourse.tile as tile
from concourse.bass_utils import run_bass_kernel_spmd

# Steer bacc's activation-table placement: drop Exp/Ln from the tables that
# hold only one of them, so any Exp or Ln resolves to the combined
# natural_log_exp_and_others table (ids keep their act_info.json positions,
# so the correct hardware table is still loaded). Without this the pass
# thrashes 1.5us table loads on every Ln<->Exp switch.
_real_get_tables = bacc.get_activation_tables


def _patched_get_tables(arch):
    t = {k: set(v) for k, v in _real_get_tables(arch).items()}
    t["exp_and_others"].discard(mybir.ActivationFunctionType.Exp)
    t["exp_and_friends"].discard(mybir.ActivationFunctionType.Exp)
    t["natural_log"].discard(mybir.ActivationFunctionType.Ln)
    return t


bacc.get_activation_tables = _patched_get_tables

F32 = mybir.dt.float32
F32R = mybir.dt.float32r
AX = mybir.AxisListType
AF = mybir.ActivationFunctionType
ALU = mybir.AluOpType

N_CORES = 8
N, M, D = 1024, 65536, 128
M_SHARD = M // N_CORES          # 8192 memory-bank rows per core
N_SHARD = N // N_CORES          # 128 samples per core
T_BLK = 8                       # 128-row blocks packed per big tile
BT = 128 * T_BLK                # 1024 rows per big tile
N_TILES = M_SHARD // BT         # 8 big tiles per core
INV_TEMP = 10.0                 # 1 / 0.1


def _build_a():
    """Per-core weighted row-sum of the memory-bank shard -> red [8, 128]."""
    nc = bacc.Bacc("TRN2", target_bir_lowering=False, debug=False,
                   num_devices=N_CORES)
    mem = nc.dram_tensor("mem", [M_SHARD, D], F32, kind="ExternalInput").ap()
    real = nc.dram_tensor("real", [N_SHARD, D], F32, kind="ExternalInput").ap()
    pert = nc.dram_tensor("pert", [N_SHARD, D], F32, kind="ExternalInput").ap()
    red_out = nc.dram_tensor(
        "red", [T_BLK, T_BLK * D], F32, kind="ExternalOutput"
    ).ap()
    stats_out = nc.dram_tensor(
        "stats", [N_SHARD, 2], F32, kind="ExternalOutput"
    ).ap()

    with tile.TileContext(nc) as tc:
        with (
            tc.tile_pool(name="mem_pool", bufs=N_TILES) as mem_pool,
            tc.tile_pool(name="sq_pool", bufs=3) as sq_pool,
            tc.tile_pool(name="stat_pool", bufs=4) as stat_pool,
            tc.tile_pool(name="misc", bufs=1) as misc,
            tc.tile_pool(name="psum_acc", bufs=1, space="PSUM") as psum_acc,
        ):
            # Prefetch the Square/Sqrt activation tables during the DMA fill.
            c1 = misc.tile([1, 1], F32)
            nc.gpsimd.memset(c1[:], 1.0)
            dsq = misc.tile([1, 1], F32)
            nc.scalar.square(dsq[:], c1[:])
            dsr = misc.tile([1, 1], F32)
            nc.scalar.sqrt(dsr[:], c1[:])

            # psum_st[m, (t, d)] += sum_p inv[p, m] * x[p, t, d]; only the
            # m == t slices are wanted, the mask kills the rest. float32r
            # runs the PE at full rate (fp32 pays a LOW/HIGH double pass).
            psum_st = psum_acc.tile([T_BLK, T_BLK * D], F32)
            HALF = T_BLK // 2
            for j in range(N_TILES):
                mt = mem_pool.tile([128, T_BLK, D], F32R, tag="mt")
                src = mem[j * BT : (j + 1) * BT, :].rearrange(
                    "(p t) d -> p t d", p=128
                )
                nc.sync.dma_start(out=mt[:], in_=src.bitcast(F32R))

                sq = sq_pool.tile([128, T_BLK, D], F32, tag="sq")
                nc.scalar.square(sq[:], mt[:].bitcast(F32))
                ssq = stat_pool.tile([128, T_BLK], F32, tag="ssq")
                nc.vector.reduce_sum(ssq[:], sq[:], axis=AX.X)
                nrm = stat_pool.tile([128, T_BLK], F32, tag="nrm")
                nc.scalar.sqrt(nrm[:], ssq[:])
                inv = stat_pool.tile([128, T_BLK], F32R, tag="inv")
                with nc.allow_low_precision(reason="float32r bit-identical"):
                    nc.vector.reciprocal(inv[:], nrm[:])

                for h in range(2):
                    nc.tensor.matmul(
                        out=psum_st[:, h * HALF * D : (h + 1) * HALF * D],
                        lhsT=inv[:],
                        rhs=mt[:, h * HALF : (h + 1) * HALF, :],
                        start=(j == 0),
                        stop=(j == N_TILES - 1),
                    )

            # CE pre-stats for this core's sample slice; these slot into
            # pipeline bubbles. posv = dot_rp/(|r||p|), e_r = 1/|r|.
            real_sb = misc.tile([N_SHARD, D], F32)
            nc.sync.dma_start(out=real_sb[:], in_=real[:])
            pert_sb = misc.tile([N_SHARD, D], F32)
            nc.sync.dma_start(out=pert_sb[:], in_=pert[:])
            scr_rp = misc.tile([N_SHARD, D], F32)
            dot_rp = misc.tile([N_SHARD, 1], F32)
            nc.vector.tensor_mul(scr_rp[:], real_sb[:], pert_sb[:])
            nc.vector.reduce_sum(dot_rp[:], scr_rp[:], axis=AX.X)
            scr_r2 = misc.tile([N_SHARD, D], F32)
            ssq_r = misc.tile([N_SHARD, 1], F32)
            nc.scalar.activation(
                scr_r2[:], real_sb[:], AF.Square, accum_out=ssq_r[:]
            )
            scr_p2 = misc.tile([N_SHARD, D], F32)
            ssq_p = misc.tile([N_SHARD, 1], F32)
            nc.scalar.activation(
                scr_p2[:], pert_sb[:], AF.Square, accum_out=ssq_p[:]
            )
            nr = misc.tile([N_SHARD, 1], F32)
            nc.scalar.sqrt(nr[:], ssq_r[:])
            npp = misc.tile([N_SHARD, 1], F32)
            nc.scalar.sqrt(npp[:], ssq_p[:])
            den = misc.tile([N_SHARD, 1], F32)
            nc.vector.tensor_mul(den[:], nr[:], npp[:])
            stats = misc.tile([N_SHARD, 2], F32)
            erp = misc.tile([N_SHARD, 1], F32)
            nc.vector.reciprocal(erp[:], den[:])
            nc.vector.reciprocal(stats[:, 1:2], nr[:])
            nc.vector.tensor_mul(stats[:, 0:1], dot_rp[:], erp[:])
            nc.sync.dma_start(out=stats_out[:], in_=stats[:])

            # Ship the whole [8, 1024] accumulator; the host picks the
            # m == t diagonal slices (pure indexing, no arithmetic).
            red = misc.tile([T_BLK, T_BLK * D], F32)
            nc.scalar.copy(red[:], psum_st[:])
            nc.sync.dma_start(out=red_out[:], in_=red[:])

    nc.compile()
    return nc


def _build_b():
    """CE over this core's 128 samples given all 64 partial s rows."""
    nc = bacc.Bacc("TRN2", target_bir_lowering=False, debug=False,
                   num_devices=N_CORES)
    K = N_CORES * T_BLK  # 64 partial rows
    redall = nc.dram_tensor("redall", [K, D], F32, kind="ExternalInput").ap()
    real = nc.dram_tensor("real", [N_SHARD, D], F32, kind="ExternalInput").ap()
    stats = nc.dram_tensor("stats", [N_SHARD, 2], F32, kind="ExternalInput").ap()
    out = nc.dram_tensor("out", [1, 1], F32, kind="ExternalOutput").ap()

    with tile.TileContext(nc) as tc:
        with (
            tc.tile_pool(name="ce", bufs=1) as ce,
            tc.tile_pool(name="psum", bufs=1, space="PSUM") as psum,
        ):
            # Prefetch the combined Ln/Exp activation table during DMA fill.
            c1 = ce.tile([1, 1], F32)
            nc.gpsimd.memset(c1[:], 1.0)
            dln = ce.tile([1, 1], F32)
            nc.scalar.activation(dln[:], c1[:], AF.Ln)

            onesN = ce.tile([128, 1], F32)
            nc.gpsimd.memset(onesN[:], 1.0 / float(N))
            onesKf = ce.tile([K, D], F32)
            nc.gpsimd.memset(onesKf[:], 1.0)
            onesK = ce.tile([K, D], F32R)
            with nc.allow_low_precision(reason="float32r bit-identical"):
                nc.vector.tensor_copy(onesK[:], onesKf[:])

            redall_sb = ce.tile([K, D], F32R)
            nc.sync.dma_start(out=redall_sb[:], in_=redall[:].bitcast(F32R))
            real_sb = ce.tile([N_SHARD, D], F32)
            nc.sync.dma_start(out=real_sb[:], in_=real[:])
            stats_sb = ce.tile([N_SHARD, 2], F32)
            nc.sync.dma_start(out=stats_sb[:], in_=stats[:])

            # Reduce the 64 partial rows and broadcast s to every partition.
            psum_bc = psum.tile([128, D], F32, tag="bc")
            nc.tensor.matmul(
                out=psum_bc[:], lhsT=onesK[:], rhs=redall_sb[:],
                start=True, stop=True,
            )

            scr4 = ce.tile([N_SHARD, D], F32, tag="scr", bufs=4)
            dot_rs = ce.tile([N_SHARD, 1], F32)
            nc.vector.tensor_mul(scr4[:], real_sb[:], psum_bc[:])
            nc.vector.reduce_sum(dot_rs[:], scr4[:], axis=AX.X)

            # z/T = dot_rs * e_r - posv ; stable softplus via Exp/Ln.
            negv = ce.tile([N_SHARD, 1], F32)
            nc.vector.tensor_mul(negv[:], dot_rs[:], stats_sb[:, 1:2])
            zb = ce.tile([N_SHARD, 1], F32)
            nc.vector.tensor_sub(zb[:], negv[:], stats_sb[:, 0:1])
            mx = ce.tile([N_SHARD, 1], F32)
            nc.vector.tensor_scalar(
                out=mx[:], in0=zb[:], scalar1=INV_TEMP, scalar2=0.0,
                op0=ALU.mult, op1=ALU.max,
            )
            nz = ce.tile([N_SHARD, 1], F32)
            nc.vector.tensor_scalar(
                out=nz[:], in0=zb[:], scalar1=-INV_TEMP, scalar2=0.0,
                op0=ALU.mult, op1=ALU.max,
            )
            e1 = ce.tile([N_SHARD, 1], F32)
            nc.scalar.activation(e1[:], nz[:], AF.Exp, scale=-1.0)
            e2 = ce.tile([N_SHARD, 1], F32)
            nc.scalar.activation(e2[:], mx[:], AF.Exp, scale=-1.0)
            sm = ce.tile([N_SHARD, 1], F32)
            nc.vector.tensor_add(sm[:], e1[:], e2[:])
            lg = ce.tile([N_SHARD, 1], F32)
            nc.scalar.activation(lg[:], sm[:], AF.Ln)
            loss = ce.tile([N_SHARD, 1], F32)
            nc.vector.tensor_add(loss[:], lg[:], mx[:])

            # Partition-sum of the 128 losses, pre-scaled by 1/N.
            psum_l = psum.tile([1, 1], F32, tag="l")
            nc.tensor.matmul(
                out=psum_l[:], lhsT=loss[:], rhs=onesN[:], start=True, stop=True
            )
            out_sb = ce.tile([1, 1], F32)
            nc.scalar.copy(out_sb[:], psum_l[:])
            nc.sync.dma_start(out=out[:], in_=out_sb[:])

    nc.compile()
    return nc


_CACHE = {}


def _get(name):
    if name not in _CACHE:
        _CACHE[name] = {"a": _build_a, "b": _build_b}[name]()
    return _CACHE[name]


def run(inputs, **spmd_kwargs):
    """Shard inputs, run both SPMD launches, return (scalar, results)."""
    mem = np.ascontiguousarray(np.asarray(inputs["memory_bank_features"], np.float32))
    real = np.ascontiguousarray(np.asarray(inputs["image_real_features"], np.float32))
    pert = np.ascontiguousarray(
        np.asarray(inputs["image_perturbed_features"], np.float32)
    )
    cores = list(range(N_CORES))

    in_maps_a = [
        {
            "mem": np.ascontiguousarray(mem[k * M_SHARD : (k + 1) * M_SHARD]),
            "real": np.ascontiguousarray(real[k * N_SHARD : (k + 1) * N_SHARD]),
            "pert": np.ascontiguousarray(pert[k * N_SHARD : (k + 1) * N_SHARD]),
        }
        for k in range(N_CORES)
    ]
    res_a = run_bass_kernel_spmd(_get("a"), in_maps_a, core_ids=cores, **spmd_kwargs)

    # Pure gather: pick each core's diagonal [8, 128] block (indexing only)
    # and concatenate.
    blocks = []
    for k in range(N_CORES):
        r = res_a.results[k]["red"].reshape(T_BLK, T_BLK, D)
        blocks.append(r[np.arange(T_BLK), np.arange(T_BLK)])
    redall = np.ascontiguousarray(np.concatenate(blocks, axis=0))

    in_maps_b = [
        {
            "redall": redall,
            "real": np.ascontiguousarray(real[k * N_SHARD : (k + 1) * N_SHARD]),
            "stats": np.ascontiguousarray(res_a.results[k]["stats"]),
        }
        for k in range(N_CORES)
    ]
    res_b = run_bass_kernel_spmd(_get("b"), in_maps_b, core_ids=cores, **spmd_kwargs)

    total = np.float32(0.0)
    for k in range(N_CORES):
        total += np.float32(res_b.results[k]["out"][0, 0])
    return np.asarray(total, dtype=np.float32).reshape(()), (res_a, res_b)


def kernel(**inputs) -> np.ndarray:
    value, _ = run(inputs)
    return value


# revision 22
# speedup vs baseline: 1.1095x; 1.1095x over previous
"""NT-Xent loss kernel for 8 TRN2 NeuronCores (Bass/Tile).

Math: with x = memory bank rows, the per-sample negative-similarity sum
    neg[n] = sum_m real_hat[n] . mem_hat[m] = real_hat[n] . s,
    s = sum_m mem[m] / ||mem[m]||
so the N x M GEMM collapses into a weighted row-sum of the memory bank
(one D-vector), which is purely memory-bound.

Distribution (8 cores, two SPMD launches, no collectives):
  - NEFF A: memory bank [65536,128] sharded by rows, 8192 rows/core. Each
    core computes row norms and the inv-norm-weighted row sum of its shard
    via packed float32r matmuls (M=8 "diagonal" trick + mask), emitting
    red[8,128] whose row-sum is the core's partial s.
  - Host: concatenates the 8 red blocks -> redall[64,128] (pure gather).
  - NEFF B: data-parallel CE over N. Every core reduces redall and
    broadcasts s to 128 partitions with one ones-matmul, then computes the
    stable softplus CE for its 128 samples and emits sum(loss)/N. The host
    sums the 8 partial scalars.
Cross-core start skew makes on-device collectives cost ~70us on this
runtime (44us entry barrier + ~20us mesh floor), so two launches are much
cheaper than one AllGather.
"""

import numpy as np

import concourse.bacc as bacc
import concourse.bass as bass
import concourse.mybir as mybir
import concourse.tile as tile
from concourse.bass_utils import run_bass_kernel_spmd

# Steer bacc's activation-table placement: drop Exp/Ln from the tables that
# hold only one of them, so any Exp or Ln resolves to the combined
# natural_log_exp_and_others table (ids keep their act_info.json positions,
# so the correct hardware table is still loaded). Without this the pass
# thrashes 1.5us table loads on every Ln<->Exp switch.
_real_get_tables = bacc.get_activation_tables


def _patched_get_tables(arch):
    t = {k: set(v) for k, v in _real_get_tables(arch).items()}
    t["exp_and_others"].discard(mybir.ActivationFunctionType.Exp)
    t["exp_and_friends"].discard(mybir.ActivationFunctionType.Exp)
    t["natural_log"].discard(mybir.ActivationFunctionType.Ln)
    return t


bacc.get_activation_tables = _patched_get_tables

F32 = mybir.dt.float32
F32R = mybir.dt.float32r
AX = mybir.AxisListType
AF = mybir.ActivationFunctionType
ALU = mybir.AluOpType

N_CORES = 8
N, M, D = 1024, 65536, 128
M_SHARD = M // N_CORES          # 8192 memory-bank rows per core
N_SHARD = N // N_CORES          # 128 samples per core
T_BLK = 8                       # 128-row blocks packed per big tile
BT = 128 * T_BLK                # 1024 rows per big tile
N_TILES = M_SHARD // BT         # 8 big tiles per core
INV_TEMP = 10.0                 # 1 / 0.1


def _build_a():
    """Per-core weighted row-sum of the memory-bank shard -> red [8, 128]."""
    nc = bacc.Bacc("TRN2", target_bir_lowering=False, debug=False,
                   num_devices=N_CORES)
    mem = nc.dram_tensor("mem", [M_SHARD, D], F32, kind="ExternalInput").ap()
    real = nc.dram_tensor("real", [N_SHARD, D], F32, kind="ExternalInput").ap()
    pert = nc.dram_tensor("pert", [N_SHARD, D], F32, kind="ExternalInput").ap()
    red_out = nc.dram_tensor(
        "red", [T_BLK, T_BLK * D], F32, kind="ExternalOutput"
    ).ap()
    stats_out = nc.dram_tensor(
        "stats", [N_SHARD, 2], F32, kind="ExternalOutput"
    ).ap()

    with tile.TileContext(nc) as tc:
        with (
            tc.tile_pool(name="mem_pool", bufs=N_TILES) as mem_pool,
            tc.tile_pool(name="sq_pool", bufs=3) as sq_pool,
            tc.tile_pool(name="stat_pool", bufs=4) as stat_pool,
            tc.tile_pool(name="misc", bufs=1) as misc,
            tc.tile_pool(name="psum_acc", bufs=1, space="PSUM") as psum_acc,
        ):
            # Prefetch the Square/Sqrt activation tables during the DMA fill.
            c1 = misc.tile([1, 1], F32)
            nc.gpsimd.memset(c1[:], 1.0)
            dsq = misc.tile([1, 1], F32)
            nc.scalar.square(dsq[:], c1[:])
            dsr = misc.tile([1, 1], F32)
            nc.scalar.sqrt(dsr[:], c1[:])

            # psum_st[m, (t, d)] += sum_p inv[p, m] * x[p, t, d]; only the
            # m == t slices are wanted, the mask kills the rest. float32r
            # runs the PE at full rate (fp32 pays a LOW/HIGH double pass).
            psum_st = psum_acc.tile([T_BLK, T_BLK * D], F32)
            HALF = T_BLK // 2
            for j in range(N_TILES):
                mt = mem_pool.tile([128, T_BLK, D], F32R, tag="mt")
                src = mem[j * BT : (j + 1) * BT, :].rearrange(
                    "(p t) d -> p t d", p=128
                )
                HB = T_BLK // 2
                nc.sync.dma_start(
                    out=mt[:, :HB, :], in_=src[:, :HB, :].bitcast(F32R)
                )
                nc.sync.dma_start(
                    out=mt[:, HB:, :], in_=src[:, HB:, :].bitcast(F32R)
                )

                sq = sq_pool.tile([128, T_BLK, D], F32, tag="sq")
                nc.scalar.square(sq[:], mt[:].bitcast(F32))
                ssq = stat_pool.tile([128, T_BLK], F32, tag="ssq")
                nc.vector.reduce_sum(ssq[:], sq[:], axis=AX.X)
                nrm = stat_pool.tile([128, T_BLK], F32, tag="nrm")
                nc.scalar.sqrt(nrm[:], ssq[:])
                inv = stat_pool.tile([128, T_BLK], F32R, tag="inv")
                with nc.allow_low_precision(reason="float32r bit-identical"):
                    nc.vector.reciprocal(inv[:], nrm[:])

                for h in range(2):
                    nc.tensor.matmul(
                        out=psum_st[:, h * HALF * D : (h + 1) * HALF * D],
                        lhsT=inv[:],
                        rhs=mt[:, h * HALF : (h + 1) * HALF, :],
                        start=(j == 0),
                        stop=(j == N_TILES - 1),
                    )

            # CE pre-stats for this core's sample slice; these slot into
            # pipeline bubbles. posv = dot_rp/(|r||p|), e_r = 1/|r|.
            real_sb = misc.tile([N_SHARD, D], F32)
            nc.sync.dma_start(out=real_sb[:], in_=real[:])
            pert_sb = misc.tile([N_SHARD, D], F32)
            nc.sync.dma_start(out=pert_sb[:], in_=pert[:])
            scr_rp = misc.tile([N_SHARD, D], F32)
            dot_rp = misc.tile([N_SHARD, 1], F32)
            nc.vector.tensor_mul(scr_rp[:], real_sb[:], pert_sb[:])
            nc.vector.reduce_sum(dot_rp[:], scr_rp[:], axis=AX.X)
            scr_r2 = misc.tile([N_SHARD, D], F32)
            ssq_r = misc.tile([N_SHARD, 1], F32)
            nc.scalar.activation(
                scr_r2[:], real_sb[:], AF.Square, accum_out=ssq_r[:]
            )
            scr_p2 = misc.tile([N_SHARD, D], F32)
            ssq_p = misc.tile([N_SHARD, 1], F32)
            nc.scalar.activation(
                scr_p2[:], pert_sb[:], AF.Square, accum_out=ssq_p[:]
            )
            nr = misc.tile([N_SHARD, 1], F32)
            nc.scalar.sqrt(nr[:], ssq_r[:])
            npp = misc.tile([N_SHARD, 1], F32)
            nc.scalar.sqrt(npp[:], ssq_p[:])
            den = misc.tile([N_SHARD, 1], F32)
            nc.vector.tensor_mul(den[:], nr[:], npp[:])
            stats = misc.tile([N_SHARD, 2], F32)
            erp = misc.tile([N_SHARD, 1], F32)
            nc.vector.reciprocal(erp[:], den[:])
            nc.vector.reciprocal(stats[:, 1:2], nr[:])
            nc.vector.tensor_mul(stats[:, 0:1], dot_rp[:], erp[:])
            nc.sync.dma_start(out=stats_out[:], in_=stats[:])

            # Ship the whole [8, 1024] accumulator; the host picks the
            # m == t diagonal slices (pure indexing, no arithmetic).
            red = misc.tile([T_BLK, T_BLK * D], F32)
            nc.scalar.copy(red[:], psum_st[:])
            nc.sync.dma_start(out=red_out[:], in_=red[:])

    nc.compile()
    return nc


def _build_b():
    """CE over this core's 128 samples given all 64 partial s rows."""
    nc = bacc.Bacc("TRN2", target_bir_lowering=False, debug=False,
                   num_devices=N_CORES)
    K = N_CORES * T_BLK  # 64 partial rows
    redall = nc.dram_tensor("redall", [K, D], F32, kind="ExternalInput").ap()
    real = nc.dram_tensor("real", [N_SHARD, D], F32, kind="ExternalInput").ap()
    stats = nc.dram_tensor("stats", [N_SHARD, 2], F32, kind="ExternalInput").ap()
    out = nc.dram_tensor("out", [1, 1], F32, kind="ExternalOutput").ap()

    with tile.TileContext(nc) as tc:
        with (
            tc.tile_pool(name="ce", bufs=1) as ce,
            tc.tile_pool(name="psum", bufs=1, space="PSUM") as psum,
        ):
            # Prefetch the combined Ln/Exp activation table during DMA fill.
            c1 = ce.tile([1, 1], F32)
            nc.gpsimd.memset(c1[:], 1.0)
            dln = ce.tile([1, 1], F32)
            nc.scalar.activation(dln[:], c1[:], AF.Ln)

            onesN = ce.tile([128, 1], F32)
            nc.gpsimd.memset(onesN[:], 1.0 / float(N))
            onesKf = ce.tile([K, D], F32)
            nc.gpsimd.memset(onesKf[:], 1.0)
            onesK = ce.tile([K, D], F32R)
            with nc.allow_low_precision(reason="float32r bit-identical"):
                nc.vector.tensor_copy(onesK[:], onesKf[:])

            redall_sb = ce.tile([K, D], F32R)
            nc.sync.dma_start(out=redall_sb[:], in_=redall[:].bitcast(F32R))
            real_sb = ce.tile([N_SHARD, D], F32)
            nc.sync.dma_start(out=real_sb[:], in_=real[:])
            stats_sb = ce.tile([N_SHARD, 2], F32)
            nc.sync.dma_start(out=stats_sb[:], in_=stats[:])

            # Reduce the 64 partial rows and broadcast s to every partition.
            psum_bc = psum.tile([128, D], F32, tag="bc")
            nc.tensor.matmul(
                out=psum_bc[:], lhsT=onesK[:], rhs=redall_sb[:],
                start=True, stop=True,
            )

            scr4 = ce.tile([N_SHARD, D], F32, tag="scr", bufs=4)
            dot_rs = ce.tile([N_SHARD, 1], F32)
            nc.vector.tensor_mul(scr4[:], real_sb[:], psum_bc[:])
            nc.vector.reduce_sum(dot_rs[:], scr4[:], axis=AX.X)

            # z/T = dot_rs * e_r - posv ; stable softplus via Exp/Ln.
            negv = ce.tile([N_SHARD, 1], F32)
            nc.vector.tensor_mul(negv[:], dot_rs[:], stats_sb[:, 1:2])
            zb = ce.tile([N_SHARD, 1], F32)
            nc.vector.tensor_sub(zb[:], negv[:], stats_sb[:, 0:1])
            mx = ce.tile([N_SHARD, 1], F32)
            nc.vector.tensor_scalar(
                out=mx[:], in0=zb[:], scalar1=INV_TEMP, scalar2=0.0,
                op0=ALU.mult, op1=ALU.max,
            )
            nz = ce.tile([N_SHARD, 1], F32)
            nc.vector.tensor_scalar(
                out=nz[:], in0=zb[:], scalar1=-INV_TEMP, scalar2=0.0,
                op0=ALU.mult, op1=ALU.max,
            )
            e1 = ce.tile([N_SHARD, 1], F32)
            nc.scalar.activation(e1[:], nz[:], AF.Exp, scale=-1.0)
            e2 = ce.tile([N_SHARD, 1], F32)
            nc.scalar.activation(e2[:], mx[:], AF.Exp, scale=-1.0)
            sm = ce.tile([N_SHARD, 1], F32)
            nc.vector.tensor_add(sm[:], e1[:], e2[:])
            lg = ce.tile([N_SHARD, 1], F32)
            nc.scalar.activation(lg[:], sm[:], AF.Ln)
            loss = ce.tile([N_SHARD, 1], F32)
            nc.vector.tensor_add(loss[:], lg[:], mx[:])

            # Partition-sum of the 128 losses, pre-scaled by 1/N.
            psum_l = psum.tile([1, 1], F32, tag="l")
            nc.tensor.matmul(
                out=psum_l[:], lhsT=loss[:], rhs=onesN[:], start=True, stop=True
            )
            out_sb = ce.tile([1, 1], F32)
            nc.scalar.copy(out_sb[:], psum_l[:])
            nc.sync.dma_start(out=out[:], in_=out_sb[:])

    nc.compile()
    return nc


_CACHE = {}


def _get(name):
    if name not in _CACHE:
        _CACHE[name] = {"a": _build_a, "b": _build_b}[name]()
    return _CACHE[name]


def run(inputs, **spmd_kwargs):
    """Shard inputs, run both SPMD launches, return (scalar, results)."""
    mem = np.ascontiguousarray(np.asarray(inputs["memory_bank_features"], np.float32))
    real = np.ascontiguousarray(np.asarray(inputs["image_real_features"], np.float32))
    pert = np.ascontiguousarray(
        np.asarray(inputs["image_perturbed_features"], np.float32)
    )
    cores = list(range(N_CORES))

    in_maps_a = [
        {
            "mem": np.ascontiguousarray(mem[k * M_SHARD : (k + 1) * M_SHARD]),
            "real": np.ascontiguousarray(real[k * N_SHARD : (k + 1) * N_SHARD]),
            "pert": np.ascontiguousarray(pert[k * N_SHARD : (k + 1) * N_SHARD]),
        }
        for k in range(N_CORES)
    ]
    res_a = run_bass_kernel_spmd(_get("a"), in_maps_a, core_ids=cores, **spmd_kwargs)

    # Pure gather: pick each core's diagonal [8, 128] block (indexing only)
    # and concatenate.
    blocks = []
    for k in range(N_CORES):
        r = res_a.results[k]["red"].reshape(T_BLK, T_BLK, D)
        blocks.append(r[np.arange(T_BLK), np.arange(T_BLK)])
    redall = np.ascontiguousarray(np.concatenate(blocks, axis=0))

    in_maps_b = [
        {
            "redall": redall,
            "real": np.ascontiguousarray(real[k * N_SHARD : (k + 1) * N_SHARD]),
            "stats": np.ascontiguousarray(res_a.results[k]["stats"]),
        }
        for k in range(N_CORES)
    ]
    res_b = run_bass_kernel_spmd(_get("b"), in_maps_b, core_ids=cores, **spmd_kwargs)

    total = np.float32(0.0)
    for k in range(N_CORES):
        total += np.float32(res_b.results[k]["out"][0, 0])
    return np.asarray(total, dtype=np.float32).reshape(()), (res_a, res_b)


def kernel(**inputs) -> np.ndarray:
    value, _ = run(inputs)
    return value


# revision 23
# speedup vs baseline: 1.1405x; 1.0279x over previous
"""NT-Xent loss kernel for 8 TRN2 NeuronCores (Bass/Tile).

Math: with x = memory bank rows, the per-sample negative-similarity sum
    neg[n] = sum_m real_hat[n] . mem_hat[m] = real_hat[n] . s,
    s = sum_m mem[m] / ||mem[m]||
so the N x M GEMM collapses into a weighted row-sum of the memory bank
(one D-vector), which is purely memory-bound.

Distribution (8 cores, two SPMD launches, no collectives):
  - NEFF A: memory bank [65536,128] sharded by rows, 8192 rows/core. Each
    core computes row norms and the inv-norm-weighted row sum of its shard
    via packed float32r matmuls (M=8 "diagonal" trick + mask), emitting
    red[8,128] whose row-sum is the core's partial s.
  - Host: concatenates the 8 red blocks -> redall[64,128] (pure gather).
  - NEFF B: data-parallel CE over N. Every core reduces redall and
    broadcasts s to 128 partitions with one ones-matmul, then computes the
    stable softplus CE for its 128 samples and emits sum(loss)/N. The host
    sums the 8 partial scalars.
Cross-core start skew makes on-device collectives cost ~70us on this
runtime (44us entry barrier + ~20us mesh floor), so two launches are much
cheaper than one AllGather.
"""

import numpy as np

import concourse.bacc as bacc
import concourse.bass as bass
import concourse.mybir as mybir
import concourse.tile as tile
from concourse.bass_utils import run_bass_kernel_spmd

# Steer bacc's activation-table placement: drop Exp/Ln from the tables that
# hold only one of them, so any Exp or Ln resolves to the combined
# natural_log_exp_and_others table (ids keep their act_info.json positions,
# so the correct hardware table is still loaded). Without this the pass
# thrashes 1.5us table loads on every Ln<->Exp switch.
_real_get_tables = bacc.get_activation_tables


def _patched_get_tables(arch):
    t = {k: set(v) for k, v in _real_get_tables(arch).items()}
    t["exp_and_others"].discard(mybir.ActivationFunctionType.Exp)
    t["exp_and_friends"].discard(mybir.ActivationFunctionType.Exp)
    t["natural_log"].discard(mybir.ActivationFunctionType.Ln)
    return t


bacc.get_activation_tables = _patched_get_tables

F32 = mybir.dt.float32
F32R = mybir.dt.float32r
AX = mybir.AxisListType
AF = mybir.ActivationFunctionType
ALU = mybir.AluOpType

N_CORES = 8
N, M, D = 1024, 65536, 128
M_SHARD = M // N_CORES          # 8192 memory-bank rows per core
N_SHARD = N // N_CORES          # 128 samples per core
T_BLK = 8                       # 128-row blocks packed per big tile
BT = 128 * T_BLK                # 1024 rows per big tile
N_TILES = M_SHARD // BT         # 8 big tiles per core
INV_TEMP = 10.0                 # 1 / 0.1


def _build_a():
    """Per-core weighted row-sum of the memory-bank shard -> red [8, 128]."""
    nc = bacc.Bacc("TRN2", target_bir_lowering=False, debug=False,
                   num_devices=N_CORES)
    mem = nc.dram_tensor("mem", [M_SHARD, D], F32, kind="ExternalInput").ap()
    real = nc.dram_tensor("real", [N_SHARD, D], F32, kind="ExternalInput").ap()
    pert = nc.dram_tensor("pert", [N_SHARD, D], F32, kind="ExternalInput").ap()
    red_out = nc.dram_tensor(
        "red", [T_BLK, T_BLK * D], F32, kind="ExternalOutput"
    ).ap()
    stats_out = nc.dram_tensor(
        "stats", [N_SHARD, 2], F32, kind="ExternalOutput"
    ).ap()

    with tile.TileContext(nc) as tc:
        with (
            tc.tile_pool(name="mem_pool", bufs=N_TILES) as mem_pool,
            tc.tile_pool(name="sq_pool", bufs=3) as sq_pool,
            tc.tile_pool(name="stat_pool", bufs=4) as stat_pool,
            tc.tile_pool(name="misc", bufs=1) as misc,
            tc.tile_pool(name="psum_acc", bufs=1, space="PSUM") as psum_acc,
        ):
            # Prefetch the Square/Sqrt activation tables during the DMA fill.
            c1 = misc.tile([1, 1], F32)
            nc.gpsimd.memset(c1[:], 1.0)
            dsq = misc.tile([1, 1], F32)
            nc.scalar.square(dsq[:], c1[:])
            dsr = misc.tile([1, 1], F32)
            nc.scalar.sqrt(dsr[:], c1[:])

            # psum_st[m, (t, d)] += sum_p inv[p, m] * x[p, t, d]; only the
            # m == t slices are wanted, the mask kills the rest. float32r
            # runs the PE at full rate (fp32 pays a LOW/HIGH double pass).
            psum_st = psum_acc.tile([T_BLK, T_BLK * D], F32)
            HALF = T_BLK // 2
            for j in range(N_TILES):
                mt = mem_pool.tile([128, T_BLK, D], F32R, tag="mt")
                src = mem[j * BT : (j + 1) * BT, :].rearrange(
                    "(p t) d -> p t d", p=128
                )
                nc.sync.dma_start(out=mt[:], in_=src.bitcast(F32R))

                sq = sq_pool.tile([128, T_BLK, D], F32, tag="sq")
                nc.scalar.square(sq[:], mt[:].bitcast(F32))
                ssq = stat_pool.tile([128, T_BLK], F32, tag="ssq")
                nc.vector.reduce_sum(ssq[:], sq[:], axis=AX.X)
                nrm = stat_pool.tile([128, T_BLK], F32, tag="nrm")
                nc.scalar.sqrt(nrm[:], ssq[:])
                inv = stat_pool.tile([128, T_BLK], F32R, tag="inv")
                with nc.allow_low_precision(reason="float32r bit-identical"):
                    nc.vector.reciprocal(inv[:], nrm[:])

                for h in range(2):
                    nc.tensor.matmul(
                        out=psum_st[:, h * HALF * D : (h + 1) * HALF * D],
                        lhsT=inv[:],
                        rhs=mt[:, h * HALF : (h + 1) * HALF, :],
                        start=(j == 0),
                        stop=(j == N_TILES - 1),
                    )

            # CE pre-stats for this core's sample slice; these slot into
            # pipeline bubbles. posv = dot_rp/(|r||p|), e_r = 1/|r|.
            real_sb = misc.tile([N_SHARD, D], F32)
            nc.sync.dma_start(out=real_sb[:], in_=real[:])
            pert_sb = misc.tile([N_SHARD, D], F32)
            nc.sync.dma_start(out=pert_sb[:], in_=pert[:])
            scr_rp = misc.tile([N_SHARD, D], F32)
            dot_rp = misc.tile([N_SHARD, 1], F32)
            nc.vector.tensor_mul(scr_rp[:], real_sb[:], pert_sb[:])
            nc.vector.reduce_sum(dot_rp[:], scr_rp[:], axis=AX.X)
            scr_r2 = misc.tile([N_SHARD, D], F32)
            ssq_r = misc.tile([N_SHARD, 1], F32)
            nc.scalar.activation(
                scr_r2[:], real_sb[:], AF.Square, accum_out=ssq_r[:]
            )
            scr_p2 = misc.tile([N_SHARD, D], F32)
            ssq_p = misc.tile([N_SHARD, 1], F32)
            nc.scalar.activation(
                scr_p2[:], pert_sb[:], AF.Square, accum_out=ssq_p[:]
            )
            nr = misc.tile([N_SHARD, 1], F32)
            nc.scalar.sqrt(nr[:], ssq_r[:])
            npp = misc.tile([N_SHARD, 1], F32)
            nc.scalar.sqrt(npp[:], ssq_p[:])
            den = misc.tile([N_SHARD, 1], F32)
            nc.vector.tensor_mul(den[:], nr[:], npp[:])
            stats = misc.tile([N_SHARD, 2], F32)
            erp = misc.tile([N_SHARD, 1], F32)
            nc.vector.reciprocal(erp[:], den[:])
            nc.vector.reciprocal(stats[:, 1:2], nr[:])
            nc.vector.tensor_mul(stats[:, 0:1], dot_rp[:], erp[:])
            nc.sync.dma_start(out=stats_out[:], in_=stats[:])

            # Ship the whole [8, 1024] accumulator; the host picks the
            # m == t diagonal slices (pure indexing, no arithmetic).
            red = misc.tile([T_BLK, T_BLK * D], F32)
            nc.scalar.copy(red[:], psum_st[:])
            nc.sync.dma_start(out=red_out[:], in_=red[:])

    nc.compile()
    return nc


def _build_b():
    """CE over this core's 128 samples given all 64 partial s rows."""
    nc = bacc.Bacc("TRN2", target_bir_lowering=False, debug=False,
                   num_devices=N_CORES)
    K = N_CORES * T_BLK  # 64 partial rows
    redall = nc.dram_tensor("redall", [K, D], F32, kind="ExternalInput").ap()
    real = nc.dram_tensor("real", [N_SHARD, D], F32, kind="ExternalInput").ap()
    stats = nc.dram_tensor("stats", [N_SHARD, 2], F32, kind="ExternalInput").ap()
    out = nc.dram_tensor("out", [1, 1], F32, kind="ExternalOutput").ap()

    with tile.TileContext(nc) as tc:
        with (
            tc.tile_pool(name="ce", bufs=1) as ce,
            tc.tile_pool(name="psum", bufs=1, space="PSUM") as psum,
        ):
            # Prefetch the combined Ln/Exp activation table during DMA fill.
            c1 = ce.tile([1, 1], F32)
            nc.gpsimd.memset(c1[:], 1.0)
            dln = ce.tile([1, 1], F32)
            nc.scalar.activation(dln[:], c1[:], AF.Ln)

            onesN = ce.tile([128, 1], F32)
            nc.gpsimd.memset(onesN[:], 1.0 / float(N))
            onesKf = ce.tile([K, D], F32)
            nc.gpsimd.memset(onesKf[:], 1.0)
            onesK = ce.tile([K, D], F32R)
            with nc.allow_low_precision(reason="float32r bit-identical"):
                nc.vector.tensor_copy(onesK[:], onesKf[:])

            redall_sb = ce.tile([K, D], F32R)
            nc.sync.dma_start(out=redall_sb[:], in_=redall[:].bitcast(F32R))
            real_sb = ce.tile([N_SHARD, D], F32)
            nc.sync.dma_start(out=real_sb[:], in_=real[:])
            stats_sb = ce.tile([N_SHARD, 2], F32)
            nc.sync.dma_start(out=stats_sb[:], in_=stats[:])

            # Reduce the 64 partial rows and broadcast s to every partition.
            psum_bc = psum.tile([128, D], F32, tag="bc")
            nc.tensor.matmul(
                out=psum_bc[:], lhsT=onesK[:], rhs=redall_sb[:],
                start=True, stop=True,
            )

            scr4 = ce.tile([N_SHARD, D], F32, tag="scr", bufs=4)
            dot_rs = ce.tile([N_SHARD, 1], F32)
            nc.vector.tensor_mul(scr4[:], real_sb[:], psum_bc[:])
            nc.vector.reduce_sum(dot_rs[:], scr4[:], axis=AX.X)

            # z/T = dot_rs * e_r - posv ; stable softplus via Exp/Ln.
            negv = ce.tile([N_SHARD, 1], F32)
            nc.vector.tensor_mul(negv[:], dot_rs[:], stats_sb[:, 1:2])
            zb = ce.tile([N_SHARD, 1], F32)
            nc.vector.tensor_sub(zb[:], negv[:], stats_sb[:, 0:1])
            mx = ce.tile([N_SHARD, 1], F32)
            nc.vector.tensor_scalar(
                out=mx[:], in0=zb[:], scalar1=INV_TEMP, scalar2=0.0,
                op0=ALU.mult, op1=ALU.max,
            )
            nz = ce.tile([N_SHARD, 1], F32)
            nc.vector.tensor_scalar(
                out=nz[:], in0=zb[:], scalar1=-INV_TEMP, scalar2=0.0,
                op0=ALU.mult, op1=ALU.max,
            )
            e1 = ce.tile([N_SHARD, 1], F32)
            nc.scalar.activation(e1[:], nz[:], AF.Exp, scale=-1.0)
            e2 = ce.tile([N_SHARD, 1], F32)
            nc.scalar.activation(e2[:], mx[:], AF.Exp, scale=-1.0)
            sm = ce.tile([N_SHARD, 1], F32)
            nc.vector.tensor_add(sm[:], e1[:], e2[:])
            lg = ce.tile([N_SHARD, 1], F32)
            nc.scalar.activation(lg[:], sm[:], AF.Ln)
            loss = ce.tile([N_SHARD, 1], F32)
            nc.vector.tensor_add(loss[:], lg[:], mx[:])

            # Partition-sum of the 128 losses, pre-scaled by 1/N.
            psum_l = psum.tile([1, 1], F32, tag="l")
            nc.tensor.matmul(
                out=psum_l[:], lhsT=loss[:], rhs=onesN[:], start=True, stop=True
            )
            out_sb = ce.tile([1, 1], F32)
            nc.scalar.copy(out_sb[:], psum_l[:])
            nc.sync.dma_start(out=out[:], in_=out_sb[:])

    nc.compile()
    return nc


_CACHE = {}


def _get(name):
    if name not in _CACHE:
        _CACHE[name] = {"a": _build_a, "b": _build_b}[name]()
    return _CACHE[name]


def run(inputs, **spmd_kwargs):
    """Shard inputs, run both SPMD launches, return (scalar, results)."""
    mem = np.ascontiguousarray(np.asarray(inputs["memory_bank_features"], np.float32))
    real = np.ascontiguousarray(np.asarray(inputs["image_real_features"], np.float32))
    pert = np.ascontiguousarray(
        np.asarray(inputs["image_perturbed_features"], np.float32)
    )
    cores = list(range(N_CORES))

    in_maps_a = [
        {
            "mem": np.ascontiguousarray(mem[k * M_SHARD : (k + 1) * M_SHARD]),
            "real": np.ascontiguousarray(real[k * N_SHARD : (k + 1) * N_SHARD]),
            "pert": np.ascontiguousarray(pert[k * N_SHARD : (k + 1) * N_SHARD]),
        }
        for k in range(N_CORES)
    ]
    res_a = run_bass_kernel_spmd(_get("a"), in_maps_a, core_ids=cores, **spmd_kwargs)

    # Pure gather: pick each core's diagonal [8, 128] block (indexing only)
    # and concatenate.
    blocks = []
    for k in range(N_CORES):
        r = res_a.results[k]["red"].reshape(T_BLK, T_BLK, D)
        blocks.append(r[np.arange(T_BLK), np.arange(T_BLK)])
    redall = np.ascontiguousarray(np.concatenate(blocks, axis=0))

    in_maps_b = [
        {
            "redall": redall,
            "real": np.ascontiguousarray(real[k * N_SHARD : (k + 1) * N_SHARD]),
            "stats": np.ascontiguousarray(res_a.results[k]["stats"]),
        }
        for k in range(N_CORES)
    ]
    res_b = run_bass_kernel_spmd(_get("b"), in_maps_b, core_ids=cores, **spmd_kwargs)

    total = np.float32(0.0)
    for k in range(N_CORES):
        total += np.float32(res_b.results[k]["out"][0, 0])
    return np.asarray(total, dtype=np.float32).reshape(()), (res_a, res_b)


def kernel(**inputs) -> np.ndarray:
    value, _ = run(inputs)
    return value
